# revision 52
# baseline (speedup 1.0000x reference)
"""AttentionBlock Trainium2 Bass kernel, 8-way head-parallel + row-parallel.

Strategy:
  Host: stable-sort tokens so mask==1 tokens come first (attention is
  permutation-equivariant; the multiplicative mask zeroes scores of
  mask==0 tokens, so their softmax is uniform and their attention output
  is colmean(V) -- computed by the same code path via mask folding).
  Launch 1 (head-parallel, 2 heads/core): QKV projections, transposed
  masked scores, exp (scale fused), A@V + softmax denominators via
  matmuls, normalize.  Host relayout (pure slicing).  Launch 2
  (sequence-parallel, 512 rows/core): W_o projection + bias + residual +
  LayerNorm.  Host inverse-permutes rows.

No collectives (measured 100-300us on this fabric); the cross-core
exchange is a host-side concat between the two launches.
"""

import os

import numpy as np

import concourse.bass as bass
import concourse.mybir as mybir
import concourse.tile as tile
from concourse import bacc
from concourse.bass_utils import run_bass_kernel_spmd
from concourse.masks import make_identity

F32 = mybir.dt.float32
F32R = mybir.dt.float32r
BF16 = mybir.dt.bfloat16
AF = mybir.ActivationFunctionType
ALU = mybir.AluOpType


# Matmul dtypes: plain fp32 runs LOW_HIGH dual-pass on the PE (4 cycles per
# output row). bf16 and float32r run single-pass (1 cycle/row). The bulk
# matmuls (projections, scores, A@V) use bf16 operands with fp32 PSUM
# accumulation; the softmax-normalization chain (selector broadcast of
# 1/denom) uses float32r (~1.5e-4 rounding) because its error is correlated
# across a head's output row and the denominator matmuls use fp32 because
# f32r cannot target partition-offset PSUM destinations (ISA check).
# End-to-end output error vs the fp32 reference: ~7e-5 relative.

S, H, NH, D = 4096, 1024, 16, 64
N_CORES = 8
DCORE = H // N_CORES          # 128 head-dims per core (2 heads)
SROW = S // N_CORES           # 512 sequence rows per core in launch 2
LN_EPS = 1e-5
INV_SQRT_H = 1.0 / 32.0

TRACE = False                 # set by test harness for NTFF profiling
LAST_EXEC_NS = []             # per-launch exec time when TRACE

_module_cache = {}


def _q_chunks(n, step=512):
    out = []
    q0 = 0
    while q0 < n:
        out.append((q0, min(step, n - q0)))
        q0 += step
    return out


def _build_launch1(n1p):
    """Per-core: Ot[128, S] = normalized attention output (transposed),
    for this core's two heads, in permuted token order."""
    ncl = n1p // 128                      # k chunks inside the active block
    nc = bacc.Bacc("TRN2", target_bir_lowering=False, debug=False,
                   enable_asserts=False, num_devices=N_CORES)

    xt_d = nc.dram_tensor("xt", [H, S], BF16, kind="ExternalInput").ap()
    wq_d = nc.dram_tensor("wq", [8, 128, DCORE], BF16, kind="ExternalInput").ap()
    wk_d = nc.dram_tensor("wk", [8, 128, DCORE], BF16, kind="ExternalInput").ap()
    wv_d = nc.dram_tensor("wv", [8, 128, DCORE], BF16, kind="ExternalInput").ap()
    bq_d = nc.dram_tensor("bq", [DCORE, 1], F32, kind="ExternalInput").ap()
    bk_d = nc.dram_tensor("bk", [DCORE, 1], F32, kind="ExternalInput").ap()
    bv_d = nc.dram_tensor("bv", [DCORE, 1], F32, kind="ExternalInput").ap()
    mk_d = nc.dram_tensor("mk", [1, S], BF16, kind="ExternalInput").ap()
    ot_d = nc.dram_tensor("ot", [DCORE, S], BF16, kind="ExternalOutput").ap()

    with tile.TileContext(nc) as tc:
        with tc.tile_pool(name="const", bufs=1) as const, \
             tc.tile_pool(name="big", bufs=1) as big:
            # memset can't emit f32r; stage in f32 and copy (copy rounds)
            stage = const.tile([128, 512], F32)
            nc.vector.memset(stage[:], 1.0)
            ones_row = const.tile([1, 128], BF16)
            nc.vector.memset(ones_row[:], 1.0)
            ones_col = const.tile([128, 1], F32)
            nc.vector.memset(ones_col[:], 1.0)
            ones_colb = const.tile([128, 1], BF16)
            nc.vector.memset(ones_colb[:], 1.0)
            # selector: out[d, q] = r[h(d), q]; heads' recips live at
            # partitions 0 and 32 (matching the denom matmul outputs)
            sel_f = const.tile([64, 128], F32)
            nc.vector.memset(sel_f[:], 0.0)
            nc.vector.memset(sel_f[0:1, 0:64], 1.0)
            nc.vector.memset(sel_f[32:33, 64:128], 1.0)
            sel2 = const.tile([64, 128], F32R)
            nc.vector.tensor_copy(sel2[:], sel_f[:])
            # init to 1.0: rows 1..31 stay 1.0 forever so the batched
            # reciprocal and the selector matmul never see 0 or inf
            r2 = const.tile([64, 512], F32R)
            nc.vector.tensor_copy(r2[:], stage[0:64, :])

            wq_sb = const.tile([128, 8, DCORE], BF16)
            for c in range(8):
                nc.sync.dma_start(wq_sb[:, c, :], wq_d[c])
            wk_sb = const.tile([128, 8, DCORE], BF16)
            for c in range(8):
                nc.sync.dma_start(wk_sb[:, c, :], wk_d[c])
            wv_sb = const.tile([128, 8, DCORE], BF16)
            for c in range(8):
                nc.sync.dma_start(wv_sb[:, c, :], wv_d[c])
            bq_sb = const.tile([DCORE, 1], F32)
            nc.sync.dma_start(bq_sb[:], bq_d[:])
            bk_sb = const.tile([DCORE, 1], F32)
            nc.sync.dma_start(bk_sb[:], bk_d[:])
            bv_sb = const.tile([DCORE, 1], F32)
            nc.sync.dma_start(bv_sb[:], bv_d[:])
            mk_row = const.tile([1, S], BF16)
            nc.sync.dma_start(mk_row[:], mk_d[:])
            ident = const.tile([128, 128], BF16)
            make_identity(nc, ident[:])

            # Persistent big tensors.
            qt_sb = big.tile([128, n1p], BF16)       # Q^T * mask   [d, q]
            kt_sb = big.tile([128, n1p], BF16)       # K^T * mask   [d, k]
            vt_sb = big.tile([128, S], BF16)         # V^T (+bias)  [d, k]
            v_sb = big.tile([128, 32, DCORE], BF16)  # V (+bias)    [k%128, k//128, d]
            mb_sb = big.tile([128, n1p], F32)       # mask broadcast over partitions
            ot_sb = big.tile([DCORE, S], BF16)       # output
            vs_hi = big.tile([128, 1], F32)         # sum_{k>=n1p} V[k]
            vs_nm = big.tile([128, 1], F32)         # sum_all(V) / S

            # --- stages 0-4 share one PSUM pool; sharing tags across
            # stages removes pool barriers so everything pipelines.
            with tc.tile_pool(name="xin", bufs=6) as xin, \
                 tc.tile_pool(name="est", bufs=3) as est, \
                 tc.tile_pool(name="sm", bufs=2) as sm, \
                 tc.tile_pool(name="psA", bufs=2, space="PSUM") as psA:
                # stage 0: mask broadcast over partitions
                for q0, qlen in _q_chunks(n1p):
                    pm = psA.tile([128, 512], F32, tag="d")
                    nc.tensor.matmul(pm[:, :qlen], ones_row[:],
                                     mk_row[0:1, q0:q0 + qlen],
                                     start=True, stop=True)
                    nc.vector.tensor_copy(mb_sb[:, q0:q0 + qlen], pm[:, :qlen])
                # stage 1: projections (Q^T, K^T, V^T; V via PE transpose)
                for q0, qlen in _q_chunks(S):
                    pq = psA.tile([128, 512], F32, tag="a")
                    pk = psA.tile([128, 512], F32, tag="a")
                    pv = psA.tile([128, 512], F32, tag="c")
                    in_act = q0 < n1p
                    alen = min(qlen, n1p - q0) if in_act else 0
                    for k in range(8):
                        xt_t = xin.tile([128, 512], BF16, tag="xt")
                        nc.sync.dma_start(
                            xt_t[:, :qlen],
                            xt_d[k * 128:(k + 1) * 128, q0:q0 + qlen])
                        if in_act:
                            nc.tensor.matmul(pq[:, :alen], wq_sb[:, k, :],
                                             xt_t[:, :alen],
                                             start=(k == 0), stop=(k == 7))
                            nc.tensor.matmul(pk[:, :alen], wk_sb[:, k, :],
                                             xt_t[:, :alen],
                                             start=(k == 0), stop=(k == 7))
                        nc.tensor.matmul(pv[:, :qlen], wv_sb[:, k, :],
                                         xt_t[:, :qlen],
                                         start=(k == 0), stop=(k == 7))
                    if in_act:
                        nc.vector.scalar_tensor_tensor(
                            out=qt_sb[:, q0:q0 + alen], in0=pq[:, :alen],
                            scalar=bq_sb[:], in1=mb_sb[:, q0:q0 + alen],
                            op0=ALU.add, op1=ALU.mult)
                        nc.vector.scalar_tensor_tensor(
                            out=kt_sb[:, q0:q0 + alen], in0=pk[:, :alen],
                            scalar=bk_sb[:], in1=mb_sb[:, q0:q0 + alen],
                            op0=ALU.add, op1=ALU.mult)
                    nc.vector.tensor_scalar_add(
                        out=vt_sb[:, q0:q0 + qlen], in0=pv[:, :qlen],
                        scalar1=bv_sb[:])
                    # transpose V^T chunks -> V [k, d] for the AV matmul
                    pt = psA.tile([128, 512], BF16, tag="d")
                    for j in range(qlen // 128):
                        nc.tensor.matmul(
                            pt[:, j * 128:(j + 1) * 128],
                            vt_sb[:, q0 + j * 128:q0 + (j + 1) * 128],
                            ident[:], is_transpose=True,
                            start=(j == 0), stop=(j == qlen // 128 - 1))
                    kc0 = q0 // 128
                    nc.vector.tensor_copy(
                        out=v_sb[:, kc0:kc0 + qlen // 128, :],
                        in_=pt[:, :qlen].rearrange("p (j m) -> p j m", m=128))

                # V column sums (lo = active block, hi = tail).
                nc.vector.tensor_reduce(
                    out=vs_nm[:], in_=vt_sb[:, :n1p],
                    axis=mybir.AxisListType.X, op=ALU.add)
                if n1p < S:
                    nc.vector.tensor_reduce(
                        out=vs_hi[:], in_=vt_sb[:, n1p:],
                        axis=mybir.AxisListType.X, op=ALU.add)
                else:
                    nc.vector.memset(vs_hi[:], 0.0)
                # vs_nm = (lo + hi) / S
                nc.vector.tensor_scalar(
                    out=vs_nm[:], in0=vs_nm[:], scalar1=vs_hi[:],
                    scalar2=1.0 / S, op0=ALU.add, op1=ALU.mult)

                # stages 2-4: scores -> exp sweep, then AV + denom burst
                for q0, qlen in _q_chunks(n1p):
                    pot = psA.tile([128, 512], F32, tag="c")
                    e_big = {}
                    for h in (0, 1):
                        e_big[h] = est.tile([128, ncl, 512], BF16,
                                            tag=f"e{h}", name=f"ebig{h}")
                    nbund = (ncl + 1) // 2
                    for b in range(nbund):
                        kcs = list(range(b * 2, min(b * 2 + 2, ncl)))
                        nj = len(kcs)
                        for h in (0, 1):
                            pst = psA.tile([128, 2, 512], F32, tag="a",
                                           name=f"pst{h}")
                            for j, kc in enumerate(kcs):
                                nc.tensor.matmul(
                                    pst[:, j, :qlen],
                                    kt_sb[64 * h:64 * (h + 1),
                                          kc * 128:(kc + 1) * 128],
                                    qt_sb[64 * h:64 * (h + 1), q0:q0 + qlen],
                                    start=True, stop=True,
                                    tile_position=(64 * h, 0))
                            nc.scalar.activation(
                                out=e_big[h][:, b * 2:b * 2 + nj, :qlen],
                                in_=pst[:, :nj, :qlen],
                                func=AF.Exp, scale=INV_SQRT_H)
                    pdn = psA.tile([128, 512], F32, tag="d")
                    for kc in range(ncl):
                        first, last = kc == 0, kc == ncl - 1
                        for h in (0, 1):
                            # partition-disjoint groups in one bank; the
                            # group checker is partition-blind (verified
                            # partition-range exec semantics in sim)
                            nc.tensor.matmul(
                                pot[64 * h:64 * (h + 1), :qlen],
                                v_sb[:, kc, 64 * h:64 * (h + 1)],
                                e_big[h][:, kc, :qlen],
                                start=first, stop=last,
                                tile_position=(0, 64 * h),
                                skip_group_check=True)
                        for h in (0, 1):
                            nc.tensor.matmul(
                                pdn[32 * h:32 * h + 1, :qlen],
                                ones_colb[:, 0:1],
                                e_big[h][:, kc, :qlen],
                                start=first, stop=last,
                                tile_position=(0, 32 * h),
                                skip_group_check=True)
                    # normalize: r = 1/(denom + (S - n1p)); broadcast over d
                    zc = float(S - n1p)
                    nc.vector.tensor_scalar_add(out=r2[0:1, :qlen],
                                                in0=pdn[0:1, :qlen],
                                                scalar1=zc)
                    nc.vector.tensor_scalar_add(out=r2[32:33, :qlen],
                                                in0=pdn[32:33, :qlen],
                                                scalar1=zc)
                    with nc.allow_low_precision(
                            reason="recip of softmax denom; f32r rounding "
                                   "(~1e-4) is far below output tolerance"):
                        # rows 1..31 hold 1.0 so one batched call is safe
                        nc.vector.reciprocal(r2[0:33, :qlen],
                                             r2[0:33, :qlen])
                    prb = psA.tile([128, 512], F32, tag="d")
                    nc.tensor.matmul(prb[:, :qlen], sel2[:],
                                     r2[:, :qlen], start=True, stop=True)
                    rb = sm.tile([128, 512], F32, tag="rb")
                    nc.vector.tensor_copy(rb[:, :qlen], prb[:, :qlen])
                    nc.vector.scalar_tensor_tensor(
                        out=ot_sb[:, q0:q0 + qlen], in0=pot[:, :qlen],
                        scalar=vs_hi[:], in1=rb[:, :qlen],
                        op0=ALU.add, op1=ALU.mult)

            # --- stage 5: tail rows (mask==0): colmean(V) ------------------
            if n1p < S:
                nc.vector.memset(ot_sb[:, n1p:], 1.0)
                nc.vector.tensor_scalar_mul(out=ot_sb[:, n1p:],
                                            in0=ot_sb[:, n1p:],
                                            scalar1=vs_nm[:])
            for c in range(8):
                nc.sync.dma_start(ot_d[:, c * 512:(c + 1) * 512],
                                  ot_sb[:, c * 512:(c + 1) * 512])

    nc.compile()
    return nc


def _build_launch2():
    """Per-core: rows [c*512, (c+1)*512) of W_o projection + residual + LN."""
    nc = bacc.Bacc("TRN2", target_bir_lowering=False, debug=False,
                   enable_asserts=False, num_devices=N_CORES)
    oa_d = nc.dram_tensor("oa", [8, 128, SROW], BF16, kind="ExternalInput").ap()
    xr_d = nc.dram_tensor("xr", [SROW, H], F32, kind="ExternalInput").ap()
    wo_d = nc.dram_tensor("wo", [H, H], BF16, kind="ExternalInput").ap()
    bo_d = nc.dram_tensor("bo", [1, H], F32R, kind="ExternalInput").ap()
    lw_d = nc.dram_tensor("lw", [1, H], F32R, kind="ExternalInput").ap()
    lb_d = nc.dram_tensor("lb", [1, H], F32R, kind="ExternalInput").ap()
    y_d = nc.dram_tensor("y", [SROW, H], F32, kind="ExternalOutput").ap()

    with tile.TileContext(nc) as tc:
        with tc.tile_pool(name="const", bufs=1) as const:
            eps_sb = const.tile([128, 1], F32)
            nc.vector.memset(eps_sb[:], LN_EPS)
            ones_f = const.tile([1, 128], F32)
            nc.vector.memset(ones_f[:], 1.0)
            ones_row = const.tile([1, 128], F32R)
            nc.vector.tensor_copy(ones_row[:], ones_f[:])
            oa_sb = const.tile([128, 8, SROW], BF16)
            for c in range(8):
                nc.sync.dma_start(oa_sb[:, c, :], oa_d[c])
            wo_sb = const.tile([128, 8, H], BF16)
            for c in range(8):
                nc.sync.dma_start(wo_sb[:, c, :],
                                  wo_d[c * 128:(c + 1) * 128, :])

            rows = {}
            for name, d in (("bo", bo_d), ("lw", lw_d), ("lb", lb_d)):
                r = const.tile([1, H], F32R, name=f"{name}_row")
                nc.sync.dma_start(r[:], d[:])
                rows[name] = r
            bcast = {}
            with tc.tile_pool(name="work", bufs=3) as work, \
                 tc.tile_pool(name="ps2", bufs=3, space="PSUM") as ps2:
                for name in ("bo", "lw", "lb"):
                    bc = const.tile([128, H], F32, name=f"{name}_bc")
                    for n in range(2):
                        pb = ps2.tile([128, 512], F32, tag="pb", bufs=2)
                        nc.tensor.matmul(pb[:], ones_row[:],
                                         rows[name][0:1, n * 512:(n + 1) * 512],
                                         start=True, stop=True)
                        nc.vector.tensor_copy(bc[:, n * 512:(n + 1) * 512], pb[:])
                    bcast[name] = bc
                for m in range(SROW // 128):
                    pr = ps2.tile([128, H], F32, tag="pr")
                    for n in range(2):
                        for k in range(8):
                            nc.tensor.matmul(
                                pr[:, n * 512:(n + 1) * 512],
                                oa_sb[:, k, m * 128:(m + 1) * 128],
                                wo_sb[:, k, n * 512:(n + 1) * 512],
                                start=(k == 0), stop=(k == 7))
                    xr_t = work.tile([128, H], F32, tag="xr")
                    for half in range(2):
                        nc.sync.dma_start(
                            xr_t[:, half * 512:(half + 1) * 512],
                            xr_d[m * 128:(m + 1) * 128,
                                 half * 512:(half + 1) * 512])
                    t1 = work.tile([128, H], F32, tag="t1")
                    nc.vector.tensor_tensor(out=t1[:], in0=pr[:], in1=xr_t[:],
                                            op=ALU.add)
                    nc.vector.tensor_tensor(out=t1[:], in0=t1[:],
                                            in1=bcast["bo"][:], op=ALU.add)
                    stats = work.tile([128, 2, 6], F32, tag="st")
                    t1v = t1.rearrange("p (s f) -> p s f", f=512)
                    for sg in range(2):
                        nc.vector.bn_stats(out=stats[:, sg, :], in_=t1v[:, sg, :])
                    mv = work.tile([128, 2], F32, tag="mv")
                    nc.vector.bn_aggr(out=mv[:], in_=stats[:])
                    sd = work.tile([128, 1], F32, tag="sd")
                    nc.scalar.activation(out=sd[:], in_=mv[:, 1:2],
                                         func=AF.Sqrt, bias=eps_sb[:], scale=1.0)
                    rstd = work.tile([128, 1], F32, tag="rs")
                    nc.vector.reciprocal(rstd[:], sd[:])
                    t2 = work.tile([128, H], F32, tag="t2")
                    nc.vector.tensor_scalar(
                        out=t2[:], in0=t1[:], scalar1=mv[:, 0:1],
                        scalar2=rstd[:], op0=ALU.subtract, op1=ALU.mult)
                    nc.vector.tensor_tensor(out=t2[:], in0=t2[:],
                                            in1=bcast["lw"][:], op=ALU.mult)
                    nc.vector.tensor_tensor(out=t2[:], in0=t2[:],
                                            in1=bcast["lb"][:], op=ALU.add)
                    for half in range(2):
                        nc.sync.dma_start(
                            y_d[m * 128:(m + 1) * 128,
                                half * 512:(half + 1) * 512],
                            t2[:, half * 512:(half + 1) * 512])
    nc.compile()
    return nc


def _get_modules(n1p):
    key = n1p
    if key not in _module_cache:
        _module_cache[key] = (_build_launch1(n1p), _build_launch2())
    return _module_cache[key]


def _install_ntff_hook():
    """Inject antenv.axon_hooks (missing in this image) so trace=True works."""
    import contextlib
    import ctypes
    import sys
    import types

    if "antenv.axon_hooks" in sys.modules:
        return
    lib = ctypes.CDLL("/opt/axon/libaxon_pjrt.so")
    lib.axon_start_nrt_profile.argtypes = [ctypes.POINTER(ctypes.c_int64),
                                           ctypes.c_size_t]
    lib.axon_start_nrt_profile.restype = ctypes.c_int64
    lib.axon_stop_nrt_profile.argtypes = [ctypes.c_char_p]
    lib.axon_stop_nrt_profile.restype = ctypes.c_int64

    @contextlib.contextmanager
    def _hook(output_dir, device_ids):
        import jax
        jax.devices()
        if device_ids:
            ids = (ctypes.c_int64 * len(device_ids))(*device_ids)
            rc = lib.axon_start_nrt_profile(ids, len(device_ids))
        else:
            rc = lib.axon_start_nrt_profile(None, 0)
        if rc != 0:
            raise RuntimeError(f"axon_start_nrt_profile rc={rc}")
        try:
            yield
        finally:
            lib.axon_stop_nrt_profile(str(output_dir).encode())

    mod = types.ModuleType("antenv.axon_hooks")
    mod.get_axon_ntff_profile_hook = lambda: _hook
    mod.set_axon_ntff_profile_hook = lambda h: None
    sys.modules["antenv.axon_hooks"] = mod


def _run(nc, in_maps):
    global LAST_EXEC_NS
    if TRACE:
        try:
            _install_ntff_hook()
        except Exception:
            pass
    res = run_bass_kernel_spmd(nc, in_maps, core_ids=list(range(N_CORES)),
                               trace=TRACE)
    if TRACE:
        LAST_EXEC_NS.append(res.exec_time_ns)
    return res.results


def kernel(inputs, mask, W_q, b_q, W_k, b_k, W_v, b_v, W_o, b_o, ln_w, ln_b):
    inputs = np.asarray(inputs, dtype=np.float32)
    mask = np.asarray(mask)
    global LAST_EXEC_NS
    LAST_EXEC_NS = []

    import ml_dtypes
    bf16 = ml_dtypes.bfloat16

    # Host-side shard prep: stable partition by mask (1s first).
    perm = np.argsort(-mask.astype(np.int64), kind="stable")
    n1 = int((mask != 0).sum())
    n1p = max(128, ((n1 + 127) // 128) * 128)
    xp = inputs[perm]                        # [S, H] permuted rows
    xt = np.ascontiguousarray(xp.T.astype(bf16))   # [H, S]
    mkp = np.ascontiguousarray(
        (mask[perm] != 0).astype(bf16).reshape(1, S))

    nc1, nc2 = _get_modules(n1p)

    in_maps1 = []
    for c in range(N_CORES):
        sl = slice(c * DCORE, (c + 1) * DCORE)
        in_maps1.append({
            "xt": xt,
            "wq": np.ascontiguousarray(
                W_q[:, sl].reshape(8, 128, DCORE).astype(bf16)),
            "wk": np.ascontiguousarray(
                W_k[:, sl].reshape(8, 128, DCORE).astype(bf16)),
            "wv": np.ascontiguousarray(
                W_v[:, sl].reshape(8, 128, DCORE).astype(bf16)),
            "bq": np.ascontiguousarray(b_q[sl].reshape(DCORE, 1)),
            "bk": np.ascontiguousarray(b_k[sl].reshape(DCORE, 1)),
            "bv": np.ascontiguousarray(b_v[sl].reshape(DCORE, 1)),
            "mk": mkp,
        })
    res1 = _run(nc1, in_maps1)
    ots = [r["ot"] for r in res1]            # each [128, S]

    wo = np.ascontiguousarray(np.asarray(W_o).astype(bf16))
    bo = np.ascontiguousarray(b_o.reshape(1, H))
    lw = np.ascontiguousarray(ln_w.reshape(1, H))
    lb = np.ascontiguousarray(ln_b.reshape(1, H))
    in_maps2 = []
    for c in range(N_CORES):
        qs = slice(c * SROW, (c + 1) * SROW)
        oa = np.stack([ots[k][:, qs] for k in range(N_CORES)], axis=0)
        in_maps2.append({
            "oa": np.ascontiguousarray(oa),
            "xr": np.ascontiguousarray(xp[qs]),
            "wo": wo, "bo": bo, "lw": lw, "lb": lb,
        })
    res2 = _run(nc2, in_maps2)
    yp = np.concatenate([r["y"] for r in res2], axis=0)   # [S, H] permuted
    out = np.empty_like(yp)
    out[perm] = yp
    return out
